# revision 2
# baseline (speedup 1.0000x reference)
"""LucyRNNCell fused Trainium2 kernel (8-core SPMD, batch-parallel).

Problem (hardcoded shapes): B=8, T=2048, IN_DIM=512, D=512.
    gates = (x @ W.T + b).reshape(B, T, 6, D)
    i,f,c,o,a,bg = sig/sig/tanh/sig/sig/sig of gates[...,0..5,:]
    h_t = f_t*h_{t-1} + i_t*c_t ; s_t = a_t*s_{t-1} + bg_t*h_t ; out_t = o_t*h_t
    returns (out [B,T,D], s_T [B,D])

Sharding: batch element b -> core b. W/b replicated. All device-side tensors
live in time-transposed layout [feature, time] so that
  * the 6D-gate projection is a plain K=512 matmul with W^T stationary, and
  * both linear recurrences map onto VectorE tensor_tensor_scan along the
    free (time) dimension, chained across time-chunks via initial=prev[:,-1:].
Host-side numpy does the x/W transposes while sharding and transposes the
output back while unsharding.
"""

import numpy as np

import concourse.bacc as bacc
import concourse.mybir as mybir
from concourse.tile import TileContext
from concourse.bass_utils import run_bass_kernel_spmd

B, T, IN_DIM, D = 8, 2048, 512, 512
NCORES = 8
TCH = 256            # time-chunk (free-dim) size for gate/scan tiles
NCH = T // TCH
KT = IN_DIM // 128   # contraction tiles
DT = D // 128        # hidden-dim tiles
NO = 6 * D // 128    # output-feature tiles of the gate projection

F32 = mybir.dt.float32
MM_DT = mybir.dt.float32r  # full-rate fp32 matmul mode on TRN2

# gate order in W rows: 0=i 1=f 2=c 3=o 4=a 5=bg.
# Emit in the order the vector engine consumes them.
GATE_EMIT_ORDER = [0, 2, 1, 5, 4, 3]

_NC_CACHE = {}


def _build_nc():
    nc = bacc.Bacc()
    xT = nc.dram_tensor("xT", [IN_DIM, T], MM_DT, kind="ExternalInput")
    wT = nc.dram_tensor("wT", [IN_DIM, 6 * D], MM_DT, kind="ExternalInput")
    bv = nc.dram_tensor("bv", [128, NO], F32, kind="ExternalInput")
    h0 = nc.dram_tensor("h0", [128, DT], F32, kind="ExternalInput")
    s0 = nc.dram_tensor("s0", [128, DT], F32, kind="ExternalInput")
    outT = nc.dram_tensor("outT", [D, T], F32, kind="ExternalOutput")
    sT = nc.dram_tensor("sT", [128, DT], F32, kind="ExternalOutput")

    SIG = mybir.ActivationFunctionType.Sigmoid
    TANH = mybir.ActivationFunctionType.Tanh
    MUL = mybir.AluOpType.mult
    ADD = mybir.AluOpType.add

    with TileContext(nc) as tc:
        with (
            tc.tile_pool(name="wt", bufs=1) as wt_pool,
            tc.tile_pool(name="xt", bufs=1) as xt_pool,
            tc.tile_pool(name="small", bufs=1) as small_pool,
            tc.tile_pool(name="gates", bufs=2) as gate_pool,
            tc.tile_pool(name="state", bufs=2) as state_pool,
            tc.tile_pool(name="scratch", bufs=4) as scratch_pool,
            tc.tile_pool(name="psum", bufs=8, space="PSUM") as psum_pool,
        ):
            wt_tiles = []
            for k in range(KT):
                t = wt_pool.tile([128, 6 * D], MM_DT, tag=f"wt{k}")
                nc.sync.dma_start(t[:], wT[128 * k : 128 * (k + 1), :])
                wt_tiles.append(t)
            xt_tiles = []
            for k in range(KT):
                t = xt_pool.tile([128, T], MM_DT, tag=f"xt{k}")
                nc.sync.dma_start(t[:], xT[128 * k : 128 * (k + 1), :])
                xt_tiles.append(t)
            bias_sb = small_pool.tile([128, NO], F32, tag="bias")
            nc.sync.dma_start(bias_sb[:], bv[:])
            h0_sb = small_pool.tile([128, DT], F32, tag="h0")
            nc.sync.dma_start(h0_sb[:], h0[:])
            s0_sb = small_pool.tile([128, DT], F32, tag="s0")
            nc.sync.dma_start(s0_sb[:], s0[:])
            sT_sb = small_pool.tile([128, DT], F32, tag="sTsb")

            h_prev = [None] * DT
            s_prev = [None] * DT
            for c in range(NCH):
                t0, t1 = c * TCH, (c + 1) * TCH
                gt = {}
                for g in GATE_EMIT_ORDER:
                    for jd in range(DT):
                        j = g * DT + jd
                        ps = psum_pool.tile([128, TCH], F32, tag="ps")
                        for k in range(KT):
                            nc.tensor.matmul(
                                ps[:],
                                wt_tiles[k][:, j * 128 : (j + 1) * 128],
                                xt_tiles[k][:, t0:t1],
                                start=(k == 0),
                                stop=(k == KT - 1),
                            )
                        gtile = gate_pool.tile([128, TCH], F32, tag=f"g{g}_{jd}")
                        func = TANH if g == 2 else SIG
                        nc.scalar.activation(
                            gtile[:], ps[:], func, bias=bias_sb[:, j : j + 1]
                        )
                        gt[(g, jd)] = gtile

                h_cur = [None] * DT
                s_cur = [None] * DT
                for jd in range(DT):
                    u = scratch_pool.tile([128, TCH], F32, tag="u")
                    nc.vector.tensor_mul(u[:], gt[(0, jd)][:], gt[(2, jd)][:])
                    h = state_pool.tile([128, TCH], F32, tag=f"h{jd}")
                    init = (
                        h0_sb[:, jd : jd + 1]
                        if c == 0
                        else h_prev[jd][:, TCH - 1 : TCH]
                    )
                    nc.vector.tensor_tensor_scan(
                        h[:], gt[(1, jd)][:], u[:], init, MUL, ADD
                    )
                    h_cur[jd] = h
                for jd in range(DT):
                    w = scratch_pool.tile([128, TCH], F32, tag="w")
                    nc.vector.tensor_mul(w[:], gt[(5, jd)][:], h_cur[jd][:])
                    s = state_pool.tile([128, TCH], F32, tag=f"s{jd}")
                    init = (
                        s0_sb[:, jd : jd + 1]
                        if c == 0
                        else s_prev[jd][:, TCH - 1 : TCH]
                    )
                    nc.vector.tensor_tensor_scan(
                        s[:], gt[(4, jd)][:], w[:], init, MUL, ADD
                    )
                    s_cur[jd] = s
                for jd in range(DT):
                    ot = scratch_pool.tile([128, TCH], F32, tag="ot")
                    nc.vector.tensor_mul(ot[:], gt[(3, jd)][:], h_cur[jd][:])
                    nc.sync.dma_start(outT[jd * 128 : (jd + 1) * 128, t0:t1], ot[:])
                if c == NCH - 1:
                    for jd in range(DT):
                        nc.vector.tensor_copy(
                            sT_sb[:, jd : jd + 1], s_cur[jd][:, TCH - 1 : TCH]
                        )
                    nc.sync.dma_start(sT[:], sT_sb[:])
                h_prev, s_prev = h_cur, s_cur

    nc.finalize()
    return nc


def _get_nc():
    if "nc" not in _NC_CACHE:
        _NC_CACHE["nc"] = _build_nc()
    return _NC_CACHE["nc"]


def _make_in_maps(x, h0, s0, W, b):
    x = np.ascontiguousarray(np.asarray(x, dtype=np.float32))
    h0 = np.ascontiguousarray(np.asarray(h0, dtype=np.float32))
    s0 = np.ascontiguousarray(np.asarray(s0, dtype=np.float32))
    W = np.ascontiguousarray(np.asarray(W, dtype=np.float32))
    b = np.ascontiguousarray(np.asarray(b, dtype=np.float32))

    wT = np.ascontiguousarray(W.T)                        # [IN, 6D]
    bv = np.ascontiguousarray(b.reshape(NO, 128).T)       # [128, NO]
    in_maps = []
    for i in range(NCORES):
        in_maps.append(
            {
                "xT": np.ascontiguousarray(x[i].T),       # [IN, T]
                "wT": wT,
                "bv": bv,
                "h0": np.ascontiguousarray(h0[i].reshape(DT, 128).T),
                "s0": np.ascontiguousarray(s0[i].reshape(DT, 128).T),
            }
        )
    return in_maps


def _unshard(results):
    out = np.empty((B, T, D), dtype=np.float32)
    s_T = np.empty((B, D), dtype=np.float32)
    for i in range(NCORES):
        out[i] = results[i]["outT"].T
        s_T[i] = results[i]["sT"].T.reshape(D)
    return out, s_T


def _ensure_profile_hook():
    """Register an NTFF profile hook for trace=True runs when the image's
    antenv lacks axon_hooks (degrades silently if unavailable)."""
    import sys
    import types

    try:
        from antenv.axon_hooks import get_axon_ntff_profile_hook  # noqa: F401

        return
    except ImportError:
        pass
    try:
        from trn_agent_boot.trn_boot import _ntff_profile_via_ctypes

        hook = _ntff_profile_via_ctypes("/opt/axon/libaxon_pjrt.so")
        mod = types.ModuleType("antenv.axon_hooks")
        mod.get_axon_ntff_profile_hook = lambda: hook
        mod.set_axon_ntff_profile_hook = lambda h: None
        sys.modules["antenv.axon_hooks"] = mod
    except Exception:
        pass


def run(x, h0, s0, W, b, trace=False):
    if trace:
        _ensure_profile_hook()
    nc = _get_nc()
    in_maps = _make_in_maps(x, h0, s0, W, b)
    res = run_bass_kernel_spmd(
        nc, in_maps, core_ids=list(range(NCORES)), trace=trace
    )
    out, s_T = _unshard(res.results)
    return (out, s_T), res


def kernel(x, h0, s0, W, b):
    (out, s_T), _ = run(x, h0, s0, W, b, trace=False)
    return out, s_T
